# revision 20
# baseline (speedup 1.0000x reference)
"""CMLI (cross-modal late interaction) loss on 8 Trainium2 NeuronCores.

Data-parallel over the image batch dim: core c owns image rows
[8c, 8c+8) and holds the full text features. Per owned image row i the
core computes the similarity slab sim[i, j, n, m] = <img[i,n], txt[j,m]>
with the PE (fp16 inputs, fp32 accumulate), then reduces:

  logits_per_image[i,j] = mean_n max_{valid m} sim   (mask of row i)
  logits_per_text [i,j] = mean_{valid m} max_n sim

and the per-row CE terms of both matrices on device. The host sums the
64 CE terms and assembles the [64,64] logits.

Matmul orientation: stationary = text chunk [128 D, 128 jm] (jm = 64*j+m),
moving = image rows [128 D, 392] (two image rows of 196 patches), so the
PSUM slab is [jm, n]:
  - text side max over n = free-axis reduce straight off PSUM (DVE)
  - image side max over m = ACT drain (+ additive -30000 pad mask, fp16)
    -> PE transpose -> segmented free-axis reduce (DVE)
Means over n / valid m are folded into tiny fp32 matmuls against
constant vectors (1/196 and valid/cnt masks).
"""

import numpy as np

import concourse.bass as bass
import concourse.mybir as mybir
import concourse.tile as tile
from concourse import bacc
from concourse.bass import MemorySpace
from concourse.bass_utils import run_bass_kernel_spmd
from concourse.masks import make_identity

B = 64          # batch
NP = 196        # image patches after CLS slice (197-1)
MT = 64         # text tokens after CLS slice (65-1)
D = 768         # feature dim
DC = 6          # contraction chunks of 128
NCORES = 8
RPC = B // NCORES       # image rows per core
NH = 98                 # half of the n axis (196 = 2*98)
MASK_BIAS = -30000.0    # additive pad-mask bias, fp16-safe

FP16 = mybir.dt.float16
F32 = mybir.dt.float32

_CACHE = {}


def _build_program():
    nc = bacc.Bacc(
        "TRN2", target_bir_lowering=False, debug=False, num_devices=NCORES
    )

    t_txt = nc.dram_tensor("txtT", [D, B * MT], FP16, kind="ExternalInput").ap()
    t_img = nc.dram_tensor("imgT", [D, RPC * NP], FP16, kind="ExternalInput").ap()
    t_mb = nc.dram_tensor("maskbias", [128, RPC], F32, kind="ExternalInput").ap()
    t_tm = nc.dram_tensor("textmask", [128, 2 * RPC], F32, kind="ExternalInput").ap()
    t_oh = nc.dram_tensor("onehot", [2 * RPC, B], F32, kind="ExternalInput").ap()
    o_li = nc.dram_tensor("out_li", [RPC, B], F32, kind="ExternalOutput").ap()
    o_lt = nc.dram_tensor("out_lt", [RPC, B], F32, kind="ExternalOutput").ap()
    o_ce = nc.dram_tensor("out_ce", [2 * RPC, 1], F32, kind="ExternalOutput").ap()

    JT = B * MT // 128   # 32 jm-tiles of 128
    with tile.TileContext(nc) as tc:
        with (
            tc.tile_pool(name="const", bufs=1) as cpool,
            tc.tile_pool(name="drain", bufs=3) as dpool,
            tc.tile_pool(name="mm", bufs=4, space="PSUM") as mmpool,
            tc.tile_pool(name="tr", bufs=3, space="PSUM") as trpool,
            tc.tile_pool(name="fin", bufs=1, space="PSUM") as finpool,
        ):
            # ---- persistent SBUF tiles ----
            txt_sb = cpool.tile([128, DC, B * MT], FP16, name="txt_sb")
            img_sb = cpool.tile([128, DC, RPC * NP], FP16, name="img_sb")
            mb_sb = cpool.tile([128, RPC], F32, name="mb_sb")
            tm_sb = cpool.tile([128, 2 * RPC], F32, name="tm_sb")
            oh_sb = cpool.tile([2 * RPC, B], F32, name="oh_sb")
            ident = cpool.tile([128, 128], FP16, name="ident")
            onescale = cpool.tile([NH, 1], F32, name="onescale")
            tm_all = cpool.tile([128, JT, RPC], F32, name="tm_all")
            ims = [
                cpool.tile([NH, 2, B], F32, name=f"ims{r}", tag=f"ims{r}")
                for r in range(RPC)
            ]
            li_rows = cpool.tile([RPC, 2 * B], F32, name="li_rows")
            lg = cpool.tile([2 * RPC, B], F32, name="lg")
            ce_out = cpool.tile([2 * RPC, 1], F32, name="ce_out")

            # ---- input loads ----
            rtxt = t_txt.rearrange("(d p) j -> p d j", p=128)
            rimg = t_img.rearrange("(d p) x -> p d x", p=128)
            # tiny aux tensors first — the first ACT drain needs mb_sb
            nc.sync.dma_start(out=mb_sb[:], in_=t_mb[:])
            nc.sync.dma_start(out=tm_sb[:], in_=t_tm[:])
            nc.sync.dma_start(out=oh_sb[:], in_=t_oh[:])
            # then first-needed feature slices so compute starts early;
            # issue from three different engines so descriptor generation
            # is not serialized on one sequencer
            sl0 = slice(0, 512)
            nc.sync.dma_start(out=txt_sb[:, 0, sl0], in_=rtxt[:, 0, sl0])
            nc.sync.dma_start(out=img_sb[:, 0, :], in_=rimg[:, 0, :])
            for d in range(1, DC):
                nc.sync.dma_start(out=txt_sb[:, d, sl0], in_=rtxt[:, d, sl0])
                nc.sync.dma_start(out=img_sb[:, d, :], in_=rimg[:, d, :])
            for g in range(1, 8):
                sl = slice(g * 512, (g + 1) * 512)
                nc.gpsimd.dma_start(out=txt_sb[:, :, sl], in_=rtxt[:, :, sl])
            make_identity(nc, ident)
            nc.vector.memset(onescale, 1.0 / float(NP))

            # ---- main slab loop ----
            for q in range(RPC // 2):   # image-row pairs
                for g in range(8):      # jm groups of 512 (8 j's)
                    drain = dpool.tile([128, 2, 4, NP], FP16, tag="drain")
                    for t in range(4):      # jm tiles of 128 within group
                        jmt = 4 * g + t
                        pm = mmpool.tile([128, 2 * NP], F32, tag="mm")
                        for d in range(DC):
                            nc.tensor.matmul(
                                pm,
                                txt_sb[:, d, 128 * jmt : 128 * (jmt + 1)],
                                img_sb[:, d, 392 * q : 392 * (q + 1)],
                                start=(d == 0),
                                stop=(d == DC - 1),
                            )
                        pmv = pm.rearrange("p (i n) -> p i n", i=2)
                        # drain to fp16 with pad-mask bias (image side; text
                        # side also reads this — pad columns are dropped by
                        # the zero entries of the textmask matmul)
                        for ih in range(2):
                            r = 2 * q + ih
                            nc.scalar.activation(
                                out=drain[:, ih, t, :],
                                in_=pmv[:, ih, :],
                                func=mybir.ActivationFunctionType.Identity,
                                bias=mb_sb[:, r : r + 1],
                                scale=1.0,
                            )
                    # text side: max over n for each (j,m) column, both rows
                    # and all 4 jm-tiles in one op
                    nc.vector.reduce_max(
                        out=tm_all[:, 4 * g : 4 * g + 4, 2 * q : 2 * q + 2]
                        .rearrange("p t i -> p i t"),
                        in_=drain,
                        axis=mybir.AxisListType.X,
                    )
                    for ih in range(2):
                        r = 2 * q + ih
                        tr = trpool.tile([NH, 2, 4, 128], FP16, tag="tr")
                        for t in range(4):
                            for h in range(2):
                                nc.tensor.transpose(
                                    tr[:, h, t, :],
                                    drain[:, ih, t, NH * h : NH * (h + 1)],
                                    ident,
                                )
                        trv = tr.rearrange("p h t (jp m) -> p h t jp m", jp=2)
                        nc.vector.reduce_max(
                            out=ims[r][:, :, 8 * g : 8 * g + 8],
                            in_=trv,
                            axis=mybir.AxisListType.X,
                        )

                # ---- per-row finals (overlap with next row-pair) ----
                for r in (2 * q, 2 * q + 1):
                    pli = finpool.tile([1, 2 * B], F32, tag="fin")
                    nc.tensor.matmul(
                        pli, onescale, ims[r].rearrange("p h j -> p (h j)"),
                        start=True, stop=True,
                    )
                    s_li = cpool.tile([1, 2 * B], F32, name=f"sli{r}", tag=f"sli{r}")
                    nc.scalar.copy(out=s_li, in_=pli[:, :])
                    nc.sync.dma_start(out=li_rows[r : r + 1, :], in_=s_li)

                    plt = finpool.tile([2, JT], F32, tag="fin")
                    nc.tensor.matmul(
                        plt, tm_sb[:, 2 * r : 2 * r + 2], tm_all[:, :, r],
                        start=True, stop=True,
                    )
                    s_lt = cpool.tile([2, JT], F32, name=f"slt{r}", tag=f"slt{r}")
                    nc.scalar.copy(out=s_lt, in_=plt[:, :])
                    lgt_r = lg[RPC + r : RPC + r + 1, :].rearrange(
                        "p (g2 h) -> p g2 h", h=2
                    )
                    for h in range(2):
                        nc.sync.dma_start(
                            out=lgt_r[:, :, h], in_=s_lt[h : h + 1, :]
                        )

            # ---- CE (img rows 0..7, text rows 8..15 in one batch) ----
            nc.vector.tensor_add(
                lg[0:RPC, :], li_rows[:, 0:B], li_rows[:, B : 2 * B]
            )
            nmx = cpool.tile([2 * RPC, 1], F32, name="nmx")
            se = cpool.tile([2 * RPC, 1], F32, name="se")
            lse = cpool.tile([2 * RPC, 1], F32, name="lse")
            dg = cpool.tile([2 * RPC, 1], F32, name="dg")
            ex = cpool.tile([2 * RPC, B], F32, name="ex")
            tmp = cpool.tile([2 * RPC, B], F32, name="tmp")
            nc.vector.reduce_max(out=nmx, in_=lg, axis=mybir.AxisListType.X)
            nc.vector.tensor_scalar_mul(nmx, nmx, -1.0)
            nc.scalar.activation(
                out=ex, in_=lg,
                func=mybir.ActivationFunctionType.Exp,
                bias=nmx, scale=1.0, accum_out=se,
            )
            nc.scalar.activation(
                out=lse, in_=se, func=mybir.ActivationFunctionType.Ln
            )
            nc.vector.tensor_mul(tmp, lg, oh_sb)
            nc.vector.reduce_sum(out=dg, in_=tmp, axis=mybir.AxisListType.X)
            nc.vector.tensor_sub(ce_out, lse, nmx)
            nc.vector.tensor_sub(ce_out, ce_out, dg)

            nc.sync.dma_start(out=o_li[:], in_=lg[0:RPC, :])
            nc.sync.dma_start(out=o_lt[:], in_=lg[RPC : 2 * RPC, :])
            nc.sync.dma_start(out=o_ce[:], in_=ce_out[:])

    nc.compile()
    return nc


def _host_prep(image, text, padding_masks, logit_scale):
    img = image[:, 1:, :].astype(np.float32) * np.float32(logit_scale)
    img16 = img.astype(np.float16)                      # [64,196,768]
    txt16 = text[:, 1:, :].astype(np.float16)           # [64,64,768]
    pms = padding_masks[:, 1:] != 0                     # [64,64] True = pad
    valid = (~pms).astype(np.float32)
    cnt = valid.sum(1)                                  # >= 1 (token 0 forced)

    txtT = np.ascontiguousarray(txt16.reshape(B * MT, D).T)   # [768, 4096]

    p = np.arange(128)
    in_maps = []
    for c in range(NCORES):
        rows = slice(RPC * c, RPC * (c + 1))
        imgT = np.ascontiguousarray(
            img16[rows].reshape(RPC * NP, D).T
        )                                               # [768, 1568]
        mb = np.where(
            pms[rows][:, p % 64].T, np.float32(MASK_BIAS), np.float32(0)
        ).astype(np.float32)                            # [128, 8]
        tm = np.zeros((128, 2 * RPC), np.float32)
        for r in range(RPC):
            i = RPC * c + r
            for h in range(2):
                tm[:, 2 * r + h] = ((p // 64) == h) * valid[i, p % 64] / cnt[i]
        oh = np.zeros((2 * RPC, B), np.float32)
        for r in range(RPC):
            oh[r, RPC * c + r] = 1.0
            oh[RPC + r, RPC * c + r] = 1.0
        in_maps.append(
            {
                "txtT": txtT,
                "imgT": imgT,
                "maskbias": mb,
                "textmask": tm,
                "onehot": oh,
            }
        )
    return in_maps


LAST_RESULTS = None


def kernel(image, text, padding_masks, logit_scale, _trace=False):
    global LAST_RESULTS
    if "nc" not in _CACHE:
        _CACHE["nc"] = _build_program()
    nc = _CACHE["nc"]

    in_maps = _host_prep(image, text, padding_masks, logit_scale)
    res = run_bass_kernel_spmd(
        nc, in_maps, core_ids=list(range(NCORES)), trace=_trace
    )
    LAST_RESULTS = res

    logits_img = np.concatenate(
        [res.results[c]["out_li"] for c in range(NCORES)], axis=0
    ).astype(np.float32)
    logits_txt = np.concatenate(
        [res.results[c]["out_lt"] for c in range(NCORES)], axis=0
    ).astype(np.float32)
    ce = np.stack(
        [res.results[c]["out_ce"][:, 0] for c in range(NCORES)], axis=0
    )                                                   # [8 cores, 16]
    loss = np.float32(0.5) * np.float32(
        ce[:, :RPC].mean() + ce[:, RPC:].mean()
    )
    targets = np.arange(B, dtype=np.int32)
    return np.float32(loss), logits_img, logits_txt, targets


# revision 21
# speedup vs baseline: 1.0773x; 1.0773x over previous
"""CMLI (cross-modal late interaction) loss on 8 Trainium2 NeuronCores.

Data-parallel over the image batch dim: core c owns image rows
[8c, 8c+8) and holds the full text features. Per owned image row i the
core computes the similarity slab sim[i, j, n, m] = <img[i,n], txt[j,m]>
with the PE (fp16 inputs, fp32 accumulate), then reduces:

  logits_per_image[i,j] = mean_n max_{valid m} sim   (mask of row i)
  logits_per_text [i,j] = mean_{valid m} max_n sim

and the per-row CE terms of both matrices on device. The host sums the
64 CE terms and assembles the [64,64] logits.

Matmul orientation: stationary = text chunk [128 D, 128 jm] (jm = 64*j+m),
moving = image rows [128 D, 392] (two image rows of 196 patches), so the
PSUM slab is [jm, n]:
  - text side max over n = free-axis reduce straight off PSUM (DVE)
  - image side max over m = ACT drain (+ additive -30000 pad mask, fp16)
    -> PE transpose -> segmented free-axis reduce (DVE)
Means over n / valid m are folded into tiny fp32 matmuls against
constant vectors (1/196 and valid/cnt masks).
"""

import numpy as np

import concourse.bass as bass
import concourse.mybir as mybir
import concourse.tile as tile
from concourse import bacc
from concourse.bass import MemorySpace
from concourse.bass_utils import run_bass_kernel_spmd
from concourse.masks import make_identity

B = 64          # batch
NP = 196        # image patches after CLS slice (197-1)
MT = 64         # text tokens after CLS slice (65-1)
D = 768         # feature dim
DC = 6          # contraction chunks of 128
NCORES = 8
RPC = B // NCORES       # image rows per core
NH = 98                 # half of the n axis (196 = 2*98)
MASK_BIAS = -30000.0    # additive pad-mask bias, fp16-safe

FP16 = mybir.dt.float16
F32 = mybir.dt.float32

_CACHE = {}


def _build_program():
    nc = bacc.Bacc(
        "TRN2", target_bir_lowering=False, debug=False, num_devices=NCORES
    )

    t_txt = nc.dram_tensor("txtT", [D, B * MT], FP16, kind="ExternalInput").ap()
    t_img = nc.dram_tensor("imgT", [D, RPC * NP], FP16, kind="ExternalInput").ap()
    t_mb = nc.dram_tensor("maskbias", [128, RPC], F32, kind="ExternalInput").ap()
    t_tm = nc.dram_tensor("textmask", [128, 2 * RPC], F32, kind="ExternalInput").ap()
    t_oh = nc.dram_tensor("onehot", [2 * RPC, B], F32, kind="ExternalInput").ap()
    o_li = nc.dram_tensor("out_li", [RPC, B], F32, kind="ExternalOutput").ap()
    o_lt = nc.dram_tensor("out_lt", [RPC, B], F32, kind="ExternalOutput").ap()
    o_ce = nc.dram_tensor("out_ce", [2 * RPC, 1], F32, kind="ExternalOutput").ap()

    JT = B * MT // 128   # 32 jm-tiles of 128
    with tile.TileContext(nc) as tc:
        with (
            tc.tile_pool(name="const", bufs=1) as cpool,
            tc.tile_pool(name="drain", bufs=3) as dpool,
            tc.tile_pool(name="mm", bufs=4, space="PSUM") as mmpool,
            tc.tile_pool(name="tr", bufs=3, space="PSUM") as trpool,
            tc.tile_pool(name="fin", bufs=1, space="PSUM") as finpool,
        ):
            # ---- persistent SBUF tiles ----
            txt_sb = cpool.tile([128, DC, B * MT], FP16, name="txt_sb")
            img_sb = cpool.tile([128, DC, RPC * NP], FP16, name="img_sb")
            mb_sb = cpool.tile([128, RPC], F32, name="mb_sb")
            tm_sb = cpool.tile([128, 2 * RPC], F32, name="tm_sb")
            oh_sb = cpool.tile([2 * RPC, B], F32, name="oh_sb")
            ident = cpool.tile([128, 128], FP16, name="ident")
            onescale = cpool.tile([NH, 1], F32, name="onescale")
            tm_all = cpool.tile([128, JT, RPC], F32, name="tm_all")
            ims = [
                cpool.tile([NH, 2, B], F32, name=f"ims{r}", tag=f"ims{r}")
                for r in range(RPC)
            ]
            li_rows = cpool.tile([RPC, 2 * B], F32, name="li_rows")
            lg = cpool.tile([2 * RPC, B], F32, name="lg")
            ce_out = cpool.tile([2 * RPC, 1], F32, name="ce_out")

            # ---- input loads ----
            rtxt = t_txt.rearrange("(d p) j -> p d j", p=128)
            rimg = t_img.rearrange("(d p) x -> p d x", p=128)
            # tiny aux tensors first — the first ACT drain needs mb_sb
            nc.sync.dma_start(out=mb_sb[:], in_=t_mb[:])
            nc.sync.dma_start(out=tm_sb[:], in_=t_tm[:])
            nc.sync.dma_start(out=oh_sb[:], in_=t_oh[:])
            # then first-needed feature slices so compute starts early;
            # issue from three different engines so descriptor generation
            # is not serialized on one sequencer
            sl0 = slice(0, 512)
            nc.sync.dma_start(out=txt_sb[:, 0, sl0], in_=rtxt[:, 0, sl0])
            nc.sync.dma_start(out=img_sb[:, 0, :], in_=rimg[:, 0, :])
            for d in range(1, DC):
                nc.sync.dma_start(out=txt_sb[:, d, sl0], in_=rtxt[:, d, sl0])
                nc.sync.dma_start(out=img_sb[:, d, :], in_=rimg[:, d, :])
            for g in range(1, 8):
                sl = slice(g * 512, (g + 1) * 512)
                nc.sync.dma_start(out=txt_sb[:, :, sl], in_=rtxt[:, :, sl])
            make_identity(nc, ident)
            nc.vector.memset(onescale, 1.0 / float(NP))

            # ---- main slab loop ----
            for q in range(RPC // 2):   # image-row pairs
                for g in range(8):      # jm groups of 512 (8 j's)
                    drain = dpool.tile([128, 2, 4, NP], FP16, tag="drain")
                    for t in range(4):      # jm tiles of 128 within group
                        jmt = 4 * g + t
                        pm = mmpool.tile([128, 2 * NP], F32, tag="mm")
                        for d in range(DC):
                            nc.tensor.matmul(
                                pm,
                                txt_sb[:, d, 128 * jmt : 128 * (jmt + 1)],
                                img_sb[:, d, 392 * q : 392 * (q + 1)],
                                start=(d == 0),
                                stop=(d == DC - 1),
                            )
                        pmv = pm.rearrange("p (i n) -> p i n", i=2)
                        # drain to fp16 with pad-mask bias (image side; text
                        # side also reads this — pad columns are dropped by
                        # the zero entries of the textmask matmul)
                        for ih in range(2):
                            r = 2 * q + ih
                            nc.scalar.activation(
                                out=drain[:, ih, t, :],
                                in_=pmv[:, ih, :],
                                func=mybir.ActivationFunctionType.Identity,
                                bias=mb_sb[:, r : r + 1],
                                scale=1.0,
                            )
                    # text side: max over n for each (j,m) column, both rows
                    # and all 4 jm-tiles in one op
                    nc.vector.reduce_max(
                        out=tm_all[:, 4 * g : 4 * g + 4, 2 * q : 2 * q + 2]
                        .rearrange("p t i -> p i t"),
                        in_=drain,
                        axis=mybir.AxisListType.X,
                    )
                    for ih in range(2):
                        r = 2 * q + ih
                        tr = trpool.tile([NH, 2, 4, 128], FP16, tag="tr")
                        for t in range(4):
                            for h in range(2):
                                nc.tensor.transpose(
                                    tr[:, h, t, :],
                                    drain[:, ih, t, NH * h : NH * (h + 1)],
                                    ident,
                                )
                        trv = tr.rearrange("p h t (jp m) -> p h t jp m", jp=2)
                        nc.vector.reduce_max(
                            out=ims[r][:, :, 8 * g : 8 * g + 8],
                            in_=trv,
                            axis=mybir.AxisListType.X,
                        )

                # ---- per-row finals (overlap with next row-pair) ----
                for r in (2 * q, 2 * q + 1):
                    pli = finpool.tile([1, 2 * B], F32, tag="fin")
                    nc.tensor.matmul(
                        pli, onescale, ims[r].rearrange("p h j -> p (h j)"),
                        start=True, stop=True,
                    )
                    s_li = cpool.tile([1, 2 * B], F32, name=f"sli{r}", tag=f"sli{r}")
                    nc.scalar.copy(out=s_li, in_=pli[:, :])
                    nc.sync.dma_start(out=li_rows[r : r + 1, :], in_=s_li)

                    plt = finpool.tile([2, JT], F32, tag="fin")
                    nc.tensor.matmul(
                        plt, tm_sb[:, 2 * r : 2 * r + 2], tm_all[:, :, r],
                        start=True, stop=True,
                    )
                    s_lt = cpool.tile([2, JT], F32, name=f"slt{r}", tag=f"slt{r}")
                    nc.scalar.copy(out=s_lt, in_=plt[:, :])
                    lgt_r = lg[RPC + r : RPC + r + 1, :].rearrange(
                        "p (g2 h) -> p g2 h", h=2
                    )
                    for h in range(2):
                        nc.sync.dma_start(
                            out=lgt_r[:, :, h], in_=s_lt[h : h + 1, :]
                        )

            # ---- CE (img rows 0..7, text rows 8..15 in one batch) ----
            nc.vector.tensor_add(
                lg[0:RPC, :], li_rows[:, 0:B], li_rows[:, B : 2 * B]
            )
            nmx = cpool.tile([2 * RPC, 1], F32, name="nmx")
            se = cpool.tile([2 * RPC, 1], F32, name="se")
            lse = cpool.tile([2 * RPC, 1], F32, name="lse")
            dg = cpool.tile([2 * RPC, 1], F32, name="dg")
            ex = cpool.tile([2 * RPC, B], F32, name="ex")
            tmp = cpool.tile([2 * RPC, B], F32, name="tmp")
            nc.vector.reduce_max(out=nmx, in_=lg, axis=mybir.AxisListType.X)
            nc.vector.tensor_scalar_mul(nmx, nmx, -1.0)
            nc.scalar.activation(
                out=ex, in_=lg,
                func=mybir.ActivationFunctionType.Exp,
                bias=nmx, scale=1.0, accum_out=se,
            )
            nc.scalar.activation(
                out=lse, in_=se, func=mybir.ActivationFunctionType.Ln
            )
            nc.vector.tensor_mul(tmp, lg, oh_sb)
            nc.vector.reduce_sum(out=dg, in_=tmp, axis=mybir.AxisListType.X)
            nc.vector.tensor_sub(ce_out, lse, nmx)
            nc.vector.tensor_sub(ce_out, ce_out, dg)

            nc.sync.dma_start(out=o_li[:], in_=lg[0:RPC, :])
            nc.sync.dma_start(out=o_lt[:], in_=lg[RPC : 2 * RPC, :])
            nc.sync.dma_start(out=o_ce[:], in_=ce_out[:])

    nc.compile()
    return nc


def _host_prep(image, text, padding_masks, logit_scale):
    img = image[:, 1:, :].astype(np.float32) * np.float32(logit_scale)
    img16 = img.astype(np.float16)                      # [64,196,768]
    txt16 = text[:, 1:, :].astype(np.float16)           # [64,64,768]
    pms = padding_masks[:, 1:] != 0                     # [64,64] True = pad
    valid = (~pms).astype(np.float32)
    cnt = valid.sum(1)                                  # >= 1 (token 0 forced)

    txtT = np.ascontiguousarray(txt16.reshape(B * MT, D).T)   # [768, 4096]

    p = np.arange(128)
    in_maps = []
    for c in range(NCORES):
        rows = slice(RPC * c, RPC * (c + 1))
        imgT = np.ascontiguousarray(
            img16[rows].reshape(RPC * NP, D).T
        )                                               # [768, 1568]
        mb = np.where(
            pms[rows][:, p % 64].T, np.float32(MASK_BIAS), np.float32(0)
        ).astype(np.float32)                            # [128, 8]
        tm = np.zeros((128, 2 * RPC), np.float32)
        for r in range(RPC):
            i = RPC * c + r
            for h in range(2):
                tm[:, 2 * r + h] = ((p // 64) == h) * valid[i, p % 64] / cnt[i]
        oh = np.zeros((2 * RPC, B), np.float32)
        for r in range(RPC):
            oh[r, RPC * c + r] = 1.0
            oh[RPC + r, RPC * c + r] = 1.0
        in_maps.append(
            {
                "txtT": txtT,
                "imgT": imgT,
                "maskbias": mb,
                "textmask": tm,
                "onehot": oh,
            }
        )
    return in_maps


LAST_RESULTS = None


def kernel(image, text, padding_masks, logit_scale, _trace=False):
    global LAST_RESULTS
    if "nc" not in _CACHE:
        _CACHE["nc"] = _build_program()
    nc = _CACHE["nc"]

    in_maps = _host_prep(image, text, padding_masks, logit_scale)
    res = run_bass_kernel_spmd(
        nc, in_maps, core_ids=list(range(NCORES)), trace=_trace
    )
    LAST_RESULTS = res

    logits_img = np.concatenate(
        [res.results[c]["out_li"] for c in range(NCORES)], axis=0
    ).astype(np.float32)
    logits_txt = np.concatenate(
        [res.results[c]["out_lt"] for c in range(NCORES)], axis=0
    ).astype(np.float32)
    ce = np.stack(
        [res.results[c]["out_ce"][:, 0] for c in range(NCORES)], axis=0
    )                                                   # [8 cores, 16]
    loss = np.float32(0.5) * np.float32(
        ce[:, :RPC].mean() + ce[:, RPC:].mean()
    )
    targets = np.arange(B, dtype=np.int32)
    return np.float32(loss), logits_img, logits_txt, targets
